# revision 15
# baseline (speedup 1.0000x reference)
"""Trainium2 Bass kernel for nn_MessageFunction (gnn_message_passing).

Computes, per edge e:
    x  = relu(e_vw @ W0.T + b0)                 # [E, 128]
    x  = relu(x @ W1.T + b1)                    # [E, 128]
    eo = (x @ W2.T + b2).reshape(E, 32, 32)     # [E, o, i]
    m  = einsum('eoi,ei->eo', eo, h_w)          # [E, 32]

Sharding: pure edge parallelism across 8 NeuronCores (E/8 = 16384 edges per
core), NNet parameters replicated.

Per-core layout strategy (all host-side pre-transposition, fp16 on-chip
matmul dtypes, fp32 PSUM accumulation):
  - L0/L1 run feature-major (hidden on partitions, edges on the free dim) in
    supertiles of 512 edges; relu+bias evictions on the scalar engine.
  - L2 runs oi-major: 8 chunks of 128 (o,i)-pairs; each chunk is computed for
    the whole 512-edge supertile into one PSUM bank.
  - The per-edge h_w multiply is a fused PSUM-evict+multiply
    (scalar_tensor_tensor) on the vector engine for half the chunks, and a
    scalar-engine copy-evict + 2x-mode tensor_tensor for the other half.
  - The i-contraction is 8 accumulating selection matmuls (0/1 weights) plus
    one small matmul for the b2 term, into PSUM m_newT [32, 512].
"""

import os
import sys
from contextlib import ExitStack

import numpy as np

sys.path.insert(0, "/opt/trn_rl_repo")

import concourse.bass as bass
import concourse.tile as tile
from concourse import bacc, mybir
from concourse._compat import with_exitstack
from concourse.bass_utils import run_bass_kernel_spmd

E = 131072
N_CORES = 8
E_CORE = E // N_CORES          # 16384
TILE_E = 128                   # edges per PE tile (e-major matmul M)
SUPER = 4                      # tiles per supertile
SUPER_E = SUPER * TILE_E       # 512
N_SUPER = E_CORE // SUPER_E    # 32
HID = 128
EF = 16
D = 32                         # D_IN == D_OUT == 32
OI = D * D                     # 1024
N_CHUNK = OI // 128            # 8

F32 = mybir.dt.float32
F16 = mybir.dt.float16

# Split of the 8 oi-chunks between the two evict paths:
# chunks [0, N_STT) -> DVE fused evict+mult; rest -> ACT evict + DVE 2x mult.
N_STT = 4


@with_exitstack
def _edge_mlp_kernel(
    ctx: ExitStack,
    tc: "tile.TileContext",
    out_mT: bass.AP,      # [32, E_CORE] fp32, o-major output
    ev_t: bass.AP,        # [N_SUPER, EF, SUPER_E] fp16  (e_vw transposed)
    hw8: bass.AP,         # [N_SUPER, 128, 2, SUPER_E] fp16 (h_w^T in pair layout)
    hwt: bass.AP,         # [N_SUPER, D, SUPER_E] fp16 (h_w^T for the b2 term)
    w0t: bass.AP,         # [EF, HID] fp16
    w1t: bass.AP,         # [HID, HID] fp16
    w2t: bass.AP,         # [HID, OI] fp16 (columns pair-reordered on host)
    scm: bass.AP,         # [128, 4*32] fp16 selection matrices (per pair)
    b2rt: bass.AP,        # [D, D] fp16  (b2.reshape(32,32).T)
    b0: bass.AP,          # [HID, 1] fp32
    b1: bass.AP,          # [HID, 1] fp32
):
    nc = tc.nc
    Relu = mybir.ActivationFunctionType.Relu
    Copy = mybir.ActivationFunctionType.Copy

    const = ctx.enter_context(tc.tile_pool(name="const", bufs=1))
    sup = ctx.enter_context(tc.tile_pool(name="sup", bufs=2))
    ypool = ctx.enter_context(tc.tile_pool(name="y", bufs=2))
    opool = ctx.enter_context(tc.tile_pool(name="o", bufs=2))
    ps_x = ctx.enter_context(tc.tile_pool(name="psx", bufs=2, space="PSUM"))
    ps_eo = ctx.enter_context(tc.tile_pool(name="pseo", bufs=2, space="PSUM"))
    ps_m = ctx.enter_context(tc.tile_pool(name="psm", bufs=2, space="PSUM"))

    # --- load constants once ---
    c_w0 = const.tile([EF, HID], F16)
    nc.sync.dma_start(c_w0[:], w0t[:])
    c_w1 = const.tile([HID, HID], F16)
    nc.sync.dma_start(c_w1[:], w1t[:])
    c_w2 = const.tile([HID, OI], F16)
    nc.sync.dma_start(c_w2[:], w2t[:])
    c_sc = const.tile([128, 4 * D], F16)
    nc.sync.dma_start(c_sc[:], scm[:])
    c_b2 = const.tile([D, D], F16)
    nc.sync.dma_start(c_b2[:], b2rt[:])
    c_b0 = const.tile([HID, 1], F32)
    nc.sync.dma_start(c_b0[:], b0[:])
    c_b1 = const.tile([HID, 1], F32)
    nc.sync.dma_start(c_b1[:], b1[:])

    for s in range(N_SUPER):
        ev = sup.tile([EF, SUPER_E], F16, tag="ev")
        nc.sync.dma_start(ev[:], ev_t[s])
        hw = sup.tile([128, 2, SUPER_E], F16, tag="hw")
        nc.sync.dma_start(hw[:], hw8[s])
        hwb = sup.tile([D, SUPER_E], F16, tag="hwb")
        nc.sync.dma_start(hwb[:], hwt[s])

        # L0: x1T[h, e] = sum_f W0T[f, h] * evT[f, e]
        x1p = ps_x.tile([HID, SUPER_E], F32, tag="xp")
        nc.tensor.matmul(x1p[:], c_w0[:], ev[:])
        x1s = sup.tile([HID, SUPER_E], F16, tag="x1s")
        nc.scalar.activation(x1s[:], x1p[:], Relu, bias=c_b0[:])

        # L1: x2T[h2, e] = sum_h W1T[h, h2] * x1T[h, e]
        x2p = ps_x.tile([HID, SUPER_E], F32, tag="xp")
        nc.tensor.matmul(x2p[:], c_w1[:], x1s[:])
        x2s = sup.tile([HID, SUPER_E], F16, tag="x2s")
        nc.scalar.activation(x2s[:], x2p[:], Relu, bias=c_b1[:])

        # b2 term: m_newT[o, e] = sum_i b2r[o, i] * hwT[i, e]  (accum start)
        mp = ps_m.tile([D, SUPER_E], F32, tag="mp")
        nc.tensor.matmul(
            mp[:], c_b2[:], hwb[:], start=True, stop=False
        )

        # L2 + h_w multiply, processed as chunk pairs. A pair covers o in
        # [8p, 8p+8) with i split in halves between its two chunks (W2
        # columns are pair-reordered on the host), so the two chunks'
        # y's fold with a plain elementwise add — done by an accumulate
        # DMA — and one selection matmul reduces the folded pair.
        for p in range(N_CHUNK // 2):
            c0 = 2 * p
            eo = ps_eo.tile([128, 2, SUPER_E], F32, tag="eo")
            nc.tensor.matmul(eo[:, 0, :], c_w2[:, c0 * 128 : (c0 + 1) * 128], x2s[:])
            nc.tensor.matmul(eo[:, 1, :], c_w2[:, (c0 + 1) * 128 : (c0 + 2) * 128], x2s[:])
            yc = ypool.tile([128, 2, SUPER_E], F16, tag=f"y{p}")
            if p < N_STT // 2:
                # fused evict+mult on DVE: y = eo * hw
                nc.vector.scalar_tensor_tensor(
                    yc[:], eo[:], 1.0, hw[:],
                    op0=mybir.AluOpType.mult, op1=mybir.AluOpType.mult,
                )
            else:
                # ACT evicts (fp32 psum -> fp16 sbuf), DVE multiplies at 2x
                eos = ypool.tile([128, 2, SUPER_E], F16, tag=f"eos{p % 2}")
                nc.scalar.activation(eos[:], eo[:], Copy)
                nc.vector.tensor_mul(yc[:], eos[:], hw[:])
            # fold the pair's i-halves with an accumulate DMA (idle engine)
            nc.gpsimd.dma_start(
                yc[:, 0, :], yc[:, 1, :], accum_op=mybir.AluOpType.add
            )
            # i-contraction: accumulate S_p.T @ y_fold into m_newT
            nc.tensor.matmul(
                mp[:], c_sc[:, p * D : (p + 1) * D], yc[:, 0, :],
                start=False, stop=(p == N_CHUNK // 2 - 1),
            )

        # evict m_newT and store
        ms = opool.tile([D, SUPER_E], F32, tag="ms")
        nc.scalar.activation(ms[:], mp[:], Copy)
        nc.sync.dma_start(out_mT[:, s * SUPER_E : (s + 1) * SUPER_E], ms[:])


def _build_bass():
    nc = bacc.Bacc("TRN2", target_bir_lowering=False, debug=False)
    d = {}
    d["ev_t"] = nc.dram_tensor("ev_t", [N_SUPER, EF, SUPER_E], F16, kind="ExternalInput")
    d["hw8"] = nc.dram_tensor("hw8", [N_SUPER, 128, 2, SUPER_E], F16, kind="ExternalInput")
    d["hwt"] = nc.dram_tensor("hwt", [N_SUPER, D, SUPER_E], F16, kind="ExternalInput")
    d["w0t"] = nc.dram_tensor("w0t", [EF, HID], F16, kind="ExternalInput")
    d["w1t"] = nc.dram_tensor("w1t", [HID, HID], F16, kind="ExternalInput")
    d["w2t"] = nc.dram_tensor("w2t", [HID, OI], F16, kind="ExternalInput")
    d["scm"] = nc.dram_tensor("scm", [128, 4 * D], F16, kind="ExternalInput")
    d["b2rt"] = nc.dram_tensor("b2rt", [D, D], F16, kind="ExternalInput")
    d["b0"] = nc.dram_tensor("b0", [HID, 1], F32, kind="ExternalInput")
    d["b1"] = nc.dram_tensor("b1", [HID, 1], F32, kind="ExternalInput")
    out = nc.dram_tensor("out_mT", [D, E_CORE], F32, kind="ExternalOutput")

    with tile.TileContext(nc) as tc:
        _edge_mlp_kernel(
            tc,
            out.ap(),
            d["ev_t"].ap(), d["hw8"].ap(), d["hwt"].ap(),
            d["w0t"].ap(), d["w1t"].ap(), d["w2t"].ap(),
            d["scm"].ap(), d["b2rt"].ap(),
            d["b0"].ap(), d["b1"].ap(),
        )
    nc.compile()
    return nc


def _prep_host_inputs(h_w, e_vw, W0, b0, W1, b1, W2, b2):
    """Build per-core input maps (all numpy, cheap)."""
    # shared (replicated) parameters
    w0t = np.ascontiguousarray(W0.T).astype(np.float16)            # [16, 128]
    w1t = np.ascontiguousarray(W1.T).astype(np.float16)            # [128, 128]
    # W2 columns in pair layout: pair p covers o in [8p, 8p+8); its two
    # chunks take i in [0,16) and [16,32). Within a chunk, partition
    # index = (o - 8p)*16 + (i mod 16).
    w2v = W2.reshape(D, D, HID)                                     # [o, i, h]
    cols = np.empty((OI,), np.int64)
    for p in range(4):
        for j in range(2):
            o = np.repeat(np.arange(8 * p, 8 * p + 8), 16)          # [128]
            i = np.tile(np.arange(16 * j, 16 * j + 16), 8)          # [128]
            cols[(2 * p + j) * 128 : (2 * p + j + 1) * 128] = o * D + i
    w2t = np.ascontiguousarray(W2.T[:, cols]).astype(np.float16)    # [128, 1024]
    b2r = b2.reshape(D, D)                                          # [o, i]
    b2rt = np.ascontiguousarray(b2r.T).astype(np.float16)           # [i, o]
    b0c = np.ascontiguousarray(b0.reshape(HID, 1)).astype(np.float32)
    b1c = np.ascontiguousarray(b1.reshape(HID, 1)).astype(np.float32)
    # selection matrices: scm[q, p*32 + o] = 1 iff o == 8p + q//16
    scm = np.zeros((128, 4 * D), np.float16)
    q = np.arange(128)
    for p in range(4):
        scm[q, p * D + 8 * p + q // 16] = 1.0

    in_maps = []
    for core in range(N_CORES):
        sl = slice(core * E_CORE, (core + 1) * E_CORE)
        ev_c = e_vw[sl]                                             # [16384, 16]
        hw_c = h_w[sl]                                              # [16384, 32]
        # ev_t[s, f, t*128+e] = ev_c[s*512 + t*128 + e, f]
        ev_t = np.ascontiguousarray(
            ev_c.reshape(N_SUPER, SUPER_E, EF).transpose(0, 2, 1)
        ).astype(np.float16)
        hw_t = hw_c.reshape(N_SUPER, SUPER_E, D).transpose(0, 2, 1)  # [Ns, 32, 512]
        # hw8[s, q, j, e] = hwT[s, 16*j + q%16, e]
        hw8 = np.empty((N_SUPER, 128, 2, SUPER_E), np.float16)
        qm = np.arange(128) % 16
        hw8[:, :, 0, :] = hw_t[:, qm, :]
        hw8[:, :, 1, :] = hw_t[:, 16 + qm, :]
        hwt = np.ascontiguousarray(hw_t).astype(np.float16)          # [Ns, 32, 512]
        in_maps.append({
            "ev_t": ev_t, "hw8": hw8, "hwt": hwt,
            "w0t": w0t, "w1t": w1t, "w2t": w2t,
            "scm": scm, "b2rt": b2rt, "b0": b0c, "b1": b1c,
        })
    return in_maps


_CACHE = {}


def kernel(h_v, h_w, e_vw, W0, b0, W1, b1, W2, b2, _trace=False, _results=None):
    # h_v is unused by the reference computation (only its trailing dim of 1
    # matters there); the message depends on h_w, e_vw and the NNet params.
    del h_v
    in_maps = _prep_host_inputs(
        np.asarray(h_w, np.float32), np.asarray(e_vw, np.float32),
        np.asarray(W0, np.float32), np.asarray(b0, np.float32),
        np.asarray(W1, np.float32), np.asarray(b1, np.float32),
        np.asarray(W2, np.float32), np.asarray(b2, np.float32),
    )
    if "nc" not in _CACHE:
        _CACHE["nc"] = _build_bass()
    nc = _CACHE["nc"]
    res = run_bass_kernel_spmd(
        nc, in_maps, core_ids=list(range(N_CORES)), trace=_trace,
    )
    if _results is not None:
        _results.append(res)
    parts = [res.results[c]["out_mT"] for c in range(N_CORES)]
    full_T = np.concatenate(parts, axis=1)          # [32, E]
    return np.ascontiguousarray(full_T.T)           # [E, 32]


if __name__ == "__main__":
    import reference
    inputs = reference.setup_inputs()
    inputs = {k: np.asarray(v) for k, v in inputs.items()}
    expected = np.asarray(reference.reference(**inputs))
    actual = kernel(**inputs)
    err = np.abs(actual - expected)
    denom = np.abs(expected).max()
    print("max abs err:", err.max(), "rel err:", err.max() / denom)


# revision 19
# speedup vs baseline: 1.1846x; 1.1846x over previous
"""Trainium2 Bass kernel for nn_MessageFunction (gnn_message_passing).

Computes, per edge e:
    x  = relu(e_vw @ W0.T + b0)                 # [E, 128]
    x  = relu(x @ W1.T + b1)                    # [E, 128]
    eo = (x @ W2.T + b2).reshape(E, 32, 32)     # [E, o, i]
    m  = einsum('eoi,ei->eo', eo, h_w)          # [E, 32]

Sharding: pure edge parallelism across 8 NeuronCores (E/8 = 16384 edges per
core), NNet parameters replicated.

Per-core layout strategy (all host-side pre-transposition, fp16 on-chip
matmul dtypes, fp32 PSUM accumulation):
  - L0/L1 run feature-major (hidden on partitions, edges on the free dim) in
    supertiles of 512 edges; relu+bias evictions on the scalar engine.
  - L2 runs oi-major: 8 chunks of 128 (o,i)-pairs; each chunk is computed for
    the whole 512-edge supertile into one PSUM bank.
  - The per-edge h_w multiply is a fused PSUM-evict+multiply
    (scalar_tensor_tensor) on the vector engine for half the chunks, and a
    scalar-engine copy-evict + 2x-mode tensor_tensor for the other half.
  - The i-contraction is 8 accumulating selection matmuls (0/1 weights) plus
    one small matmul for the b2 term, into PSUM m_newT [32, 512].
"""

import os
import sys
from contextlib import ExitStack

import numpy as np

sys.path.insert(0, "/opt/trn_rl_repo")

import concourse.bass as bass
import concourse.tile as tile
from concourse import bacc, mybir
from concourse._compat import with_exitstack
from concourse.bass_utils import run_bass_kernel_spmd

E = 131072
N_CORES = 8
E_CORE = E // N_CORES          # 16384
TILE_E = 128                   # edges per PE tile (e-major matmul M)
SUPER = 4                      # tiles per supertile
SUPER_E = SUPER * TILE_E       # 512
N_SUPER = E_CORE // SUPER_E    # 32
HID = 128
EF = 16
D = 32                         # D_IN == D_OUT == 32
OI = D * D                     # 1024
N_CHUNK = OI // 128            # 8

F32 = mybir.dt.float32
F16 = mybir.dt.float16

# Split of the 8 oi-chunks between the two evict paths:
# chunks [0, N_STT) -> DVE fused evict+mult; rest -> ACT evict + DVE 2x mult.
N_STT = 6


@with_exitstack
def _edge_mlp_kernel(
    ctx: ExitStack,
    tc: "tile.TileContext",
    out_mT: bass.AP,      # [32, E_CORE] fp32, o-major output
    ev_t: bass.AP,        # [N_SUPER, EF, SUPER_E] fp16  (e_vw transposed)
    hw8: bass.AP,         # [N_SUPER, 128, 2, SUPER_E] fp16 (h_w^T in pair layout)
    hwt: bass.AP,         # [N_SUPER, D, SUPER_E] fp16 (h_w^T for the b2 term)
    w0t: bass.AP,         # [EF, HID] fp16
    w1t: bass.AP,         # [HID, HID] fp16
    w2t: bass.AP,         # [HID, OI] fp16 (columns pair-reordered on host)
    scm: bass.AP,         # [128, 4*32] fp16 selection matrices (per pair)
    b2rt: bass.AP,        # [D, D] fp16  (b2.reshape(32,32).T)
    b0: bass.AP,          # [HID, 1] fp32
    b1: bass.AP,          # [HID, 1] fp32
):
    nc = tc.nc
    Relu = mybir.ActivationFunctionType.Relu
    Copy = mybir.ActivationFunctionType.Copy

    const = ctx.enter_context(tc.tile_pool(name="const", bufs=1))
    sup = ctx.enter_context(tc.tile_pool(name="sup", bufs=3))
    ypool = ctx.enter_context(tc.tile_pool(name="y", bufs=2))
    opool = ctx.enter_context(tc.tile_pool(name="o", bufs=2))
    ps_x = ctx.enter_context(tc.tile_pool(name="psx", bufs=1, space="PSUM"))
    ps_eo = ctx.enter_context(tc.tile_pool(name="pseo", bufs=3, space="PSUM"))
    ps_m = ctx.enter_context(tc.tile_pool(name="psm", bufs=1, space="PSUM"))

    # --- load constants once ---
    c_w0 = const.tile([EF, HID], F16)
    nc.sync.dma_start(c_w0[:], w0t[:])
    c_w1 = const.tile([HID, HID], F16)
    nc.sync.dma_start(c_w1[:], w1t[:])
    c_w2 = const.tile([HID, OI], F16)
    nc.sync.dma_start(c_w2[:], w2t[:])
    c_sc = const.tile([128, 4 * D], F16)
    nc.sync.dma_start(c_sc[:], scm[:])
    c_b2 = const.tile([D, D], F16)
    nc.sync.dma_start(c_b2[:], b2rt[:])
    c_b0 = const.tile([HID, 1], F32)
    nc.sync.dma_start(c_b0[:], b0[:])
    c_b1 = const.tile([HID, 1], F32)
    nc.sync.dma_start(c_b1[:], b1[:])

    for s in range(N_SUPER):
        ev = sup.tile([EF, SUPER_E], F16, tag="ev")
        nc.sync.dma_start(ev[:], ev_t[s])
        hw = sup.tile([128, 2, SUPER_E], F16, tag="hw")
        nc.sync.dma_start(hw[:], hw8[s])
        hwb = sup.tile([D, SUPER_E], F16, tag="hwb")
        nc.sync.dma_start(hwb[:], hwt[s])

        # L0: x1T[h, e] = sum_f W0T[f, h] * evT[f, e]
        x1p = ps_x.tile([HID, SUPER_E], F32, tag="xp")
        nc.tensor.matmul(x1p[:], c_w0[:], ev[:])
        x1s = sup.tile([HID, SUPER_E], F16, tag="x1s")
        nc.scalar.activation(x1s[:], x1p[:], Relu, bias=c_b0[:])

        # L1: x2T[h2, e] = sum_h W1T[h, h2] * x1T[h, e]
        x2p = ps_x.tile([HID, SUPER_E], F32, tag="xp")
        nc.tensor.matmul(x2p[:], c_w1[:], x1s[:])
        x2s = sup.tile([HID, SUPER_E], F16, tag="x2s")
        nc.scalar.activation(x2s[:], x2p[:], Relu, bias=c_b1[:])

        # b2 term: m_newT[o, e] = sum_i b2r[o, i] * hwT[i, e]  (accum start)
        mp = ps_m.tile([D, SUPER_E], F32, tag="mp")
        nc.tensor.matmul(
            mp[:], c_b2[:], hwb[:], start=True, stop=False
        )

        # L2 + h_w multiply, processed as chunk pairs (a pair = 2 PSUM
        # banks so evict ops run at FD=1024). The pair covers o in
        # [8p, 8p+8) with i split in halves between its two chunks (W2
        # columns pair-reordered on the host); each chunk is reduced by
        # its own selection matmul accumulating into m_newT.
        for p in range(N_CHUNK // 2):
            c0 = 2 * p
            eo = ps_eo.tile([128, 2, SUPER_E], F32, tag="eo")
            nc.tensor.matmul(eo[:, 0, :], c_w2[:, c0 * 128 : (c0 + 1) * 128], x2s[:])
            nc.tensor.matmul(eo[:, 1, :], c_w2[:, (c0 + 1) * 128 : (c0 + 2) * 128], x2s[:])
            yc = ypool.tile([128, 2, SUPER_E], F16, tag=f"y{p}")
            if c0 < N_STT:
                # fused evict+mult on DVE: y = eo * hw
                nc.vector.scalar_tensor_tensor(
                    yc[:], eo[:], 1.0, hw[:],
                    op0=mybir.AluOpType.mult, op1=mybir.AluOpType.mult,
                )
            else:
                # ACT evicts (fp32 psum -> fp16 sbuf), DVE multiplies at 2x
                eos = ypool.tile([128, 2, SUPER_E], F16, tag=f"eos{p % 2}")
                nc.scalar.activation(eos[:], eo[:], Copy)
                nc.vector.tensor_mul(yc[:], eos[:], hw[:])
            # i-contraction: one selection matmul per chunk into m_newT
            # (both chunks of the pair share the same q -> o mapping)
            for j in range(2):
                nc.tensor.matmul(
                    mp[:], c_sc[:, p * D : (p + 1) * D], yc[:, j, :],
                    start=False, stop=(c0 + j == N_CHUNK - 1),
                )

        # evict m_newT and store
        ms = opool.tile([D, SUPER_E], F32, tag="ms")
        nc.scalar.activation(ms[:], mp[:], Copy)
        nc.sync.dma_start(out_mT[:, s * SUPER_E : (s + 1) * SUPER_E], ms[:])


def _build_bass():
    nc = bacc.Bacc("TRN2", target_bir_lowering=False, debug=False)
    d = {}
    d["ev_t"] = nc.dram_tensor("ev_t", [N_SUPER, EF, SUPER_E], F16, kind="ExternalInput")
    d["hw8"] = nc.dram_tensor("hw8", [N_SUPER, 128, 2, SUPER_E], F16, kind="ExternalInput")
    d["hwt"] = nc.dram_tensor("hwt", [N_SUPER, D, SUPER_E], F16, kind="ExternalInput")
    d["w0t"] = nc.dram_tensor("w0t", [EF, HID], F16, kind="ExternalInput")
    d["w1t"] = nc.dram_tensor("w1t", [HID, HID], F16, kind="ExternalInput")
    d["w2t"] = nc.dram_tensor("w2t", [HID, OI], F16, kind="ExternalInput")
    d["scm"] = nc.dram_tensor("scm", [128, 4 * D], F16, kind="ExternalInput")
    d["b2rt"] = nc.dram_tensor("b2rt", [D, D], F16, kind="ExternalInput")
    d["b0"] = nc.dram_tensor("b0", [HID, 1], F32, kind="ExternalInput")
    d["b1"] = nc.dram_tensor("b1", [HID, 1], F32, kind="ExternalInput")
    out = nc.dram_tensor("out_mT", [D, E_CORE], F32, kind="ExternalOutput")

    with tile.TileContext(nc) as tc:
        _edge_mlp_kernel(
            tc,
            out.ap(),
            d["ev_t"].ap(), d["hw8"].ap(), d["hwt"].ap(),
            d["w0t"].ap(), d["w1t"].ap(), d["w2t"].ap(),
            d["scm"].ap(), d["b2rt"].ap(),
            d["b0"].ap(), d["b1"].ap(),
        )
    nc.compile()
    return nc


def _prep_host_inputs(h_w, e_vw, W0, b0, W1, b1, W2, b2):
    """Build per-core input maps (all numpy, cheap)."""
    # shared (replicated) parameters
    w0t = np.ascontiguousarray(W0.T).astype(np.float16)            # [16, 128]
    w1t = np.ascontiguousarray(W1.T).astype(np.float16)            # [128, 128]
    # W2 columns in pair layout: pair p covers o in [8p, 8p+8); its two
    # chunks take i in [0,16) and [16,32). Within a chunk, partition
    # index = (o - 8p)*16 + (i mod 16).
    w2v = W2.reshape(D, D, HID)                                     # [o, i, h]
    cols = np.empty((OI,), np.int64)
    for p in range(4):
        for j in range(2):
            o = np.repeat(np.arange(8 * p, 8 * p + 8), 16)          # [128]
            i = np.tile(np.arange(16 * j, 16 * j + 16), 8)          # [128]
            cols[(2 * p + j) * 128 : (2 * p + j + 1) * 128] = o * D + i
    w2t = np.ascontiguousarray(W2.T[:, cols]).astype(np.float16)    # [128, 1024]
    b2r = b2.reshape(D, D)                                          # [o, i]
    b2rt = np.ascontiguousarray(b2r.T).astype(np.float16)           # [i, o]
    b0c = np.ascontiguousarray(b0.reshape(HID, 1)).astype(np.float32)
    b1c = np.ascontiguousarray(b1.reshape(HID, 1)).astype(np.float32)
    # selection matrices: scm[q, p*32 + o] = 1 iff o == 8p + q//16
    scm = np.zeros((128, 4 * D), np.float16)
    q = np.arange(128)
    for p in range(4):
        scm[q, p * D + 8 * p + q // 16] = 1.0

    in_maps = []
    for core in range(N_CORES):
        sl = slice(core * E_CORE, (core + 1) * E_CORE)
        ev_c = e_vw[sl]                                             # [16384, 16]
        hw_c = h_w[sl]                                              # [16384, 32]
        # ev_t[s, f, t*128+e] = ev_c[s*512 + t*128 + e, f]
        ev_t = np.ascontiguousarray(
            ev_c.reshape(N_SUPER, SUPER_E, EF).transpose(0, 2, 1)
        ).astype(np.float16)
        hw_t = hw_c.reshape(N_SUPER, SUPER_E, D).transpose(0, 2, 1)  # [Ns, 32, 512]
        # hw8[s, q, j, e] = hwT[s, 16*j + q%16, e]
        hw8 = np.empty((N_SUPER, 128, 2, SUPER_E), np.float16)
        qm = np.arange(128) % 16
        hw8[:, :, 0, :] = hw_t[:, qm, :]
        hw8[:, :, 1, :] = hw_t[:, 16 + qm, :]
        hwt = np.ascontiguousarray(hw_t).astype(np.float16)          # [Ns, 32, 512]
        in_maps.append({
            "ev_t": ev_t, "hw8": hw8, "hwt": hwt,
            "w0t": w0t, "w1t": w1t, "w2t": w2t,
            "scm": scm, "b2rt": b2rt, "b0": b0c, "b1": b1c,
        })
    return in_maps


_CACHE = {}


def kernel(h_v, h_w, e_vw, W0, b0, W1, b1, W2, b2, _trace=False, _results=None):
    # h_v is unused by the reference computation (only its trailing dim of 1
    # matters there); the message depends on h_w, e_vw and the NNet params.
    del h_v
    in_maps = _prep_host_inputs(
        np.asarray(h_w, np.float32), np.asarray(e_vw, np.float32),
        np.asarray(W0, np.float32), np.asarray(b0, np.float32),
        np.asarray(W1, np.float32), np.asarray(b1, np.float32),
        np.asarray(W2, np.float32), np.asarray(b2, np.float32),
    )
    if "nc" not in _CACHE:
        _CACHE["nc"] = _build_bass()
    nc = _CACHE["nc"]
    res = run_bass_kernel_spmd(
        nc, in_maps, core_ids=list(range(N_CORES)), trace=_trace,
    )
    if _results is not None:
        _results.append(res)
    parts = [res.results[c]["out_mT"] for c in range(N_CORES)]
    full_T = np.concatenate(parts, axis=1)          # [32, E]
    return np.ascontiguousarray(full_T.T)           # [E, 32]


if __name__ == "__main__":
    import reference
    inputs = reference.setup_inputs()
    inputs = {k: np.asarray(v) for k, v in inputs.items()}
    expected = np.asarray(reference.reference(**inputs))
    actual = kernel(**inputs)
    err = np.abs(actual - expected)
    denom = np.abs(expected).max()
    print("max abs err:", err.max(), "rel err:", err.max() / denom)
